# revision 12
# baseline (speedup 1.0000x reference)
"""Bass/Trainium2 kernel for full attention: softmax(Q K^T / d_k) V.

Shapes (hardcoded): Q [8192, 128], K [8192, 128], V [8192, 128] -> out [8192, 128].
Sharding: Q rows split across 8 NeuronCores (1024 queries/core); K, V replicated.
Host passes Q^T/K^T (layout prep); V stays natural.

Per-core algorithm (transposed orientation -> no per-tile transposes needed):
  - Prelude: DMA K^T [128d, 8192m], Q^T [128d, 1024n], V stripes [128m, (c v)];
    cast f32 -> f32r (PE runs f32r matmuls at full rate for moving dim >= 256).
  - For each query tile (512 queries) and each key chunk (128 keys):
      S^T[m, n] = (K^T chunk).T @ Q^T slice        (PE, f32r, N=512)
      E^T = exp(S^T / 128)                         (ScalarE, PSUM->SBUF, f32r out)
      sums[1, n] += ones.T @ E^T                   (PE, PSUM accumulate)
      O^T[v, n]  += (V chunk as lhsT).T @ E^T      (PE, PSUM accumulate)
  - Normalize: recip(sums), broadcast across partitions via ones-outer-product
    matmul, multiply, DMA out O^T [128, 1024].
Host: gather + transpose per-core O^T -> full [8192, 128].
"""

import numpy as np

import concourse.bass as bass
import concourse.mybir as mybir
import concourse.tile as tile
from concourse.bass_utils import run_bass_kernel_spmd

N, M, D = 8192, 8192, 128
NCORES = 8
NLOC = N // NCORES            # 1024 queries per core
NT = 512                      # query tile (matmul moving free dim)
NTILES = NLOC // NT           # 2
MCHUNK = 128                  # key chunk (partition dim of S^T tiles)
NMC = M // MCHUNK             # 64
GRP = 2                       # m-chunks per exp group (PSUM banks per S^T tile)
NGRP = NMC // GRP             # 32
SCALE = 1.0 / D
WIDE = 1024                   # prelude DMA/cast stripe width

F32 = mybir.dt.float32
F32R = mybir.dt.float32r
EXP = mybir.ActivationFunctionType.Exp

TRACE = False                 # test.py sets True to capture NTFF profile
LAST_RESULT = {}              # test.py reads exec_time_ns etc.


def build():
    nc = bass.Bass()
    QT_d = nc.dram_tensor("QT", [D, NLOC], F32, kind="ExternalInput")
    KT_d = nc.dram_tensor("KT", [D, M], F32, kind="ExternalInput")
    V_d = nc.dram_tensor("V", [M, D], F32, kind="ExternalInput")
    OT_d = nc.dram_tensor("OT", [D, NLOC], F32, kind="ExternalOutput")

    V_r = V_d[:].rearrange("(c p) v -> p c v", p=128)  # [128, 64, 128] stripe view

    with tile.TileContext(nc) as tc:
        with (
            tc.tile_pool(name="const", bufs=1) as const,
            tc.tile_pool(name="big", bufs=1) as big,
            tc.tile_pool(name="et", bufs=3) as etp,
            tc.tile_pool(name="outp", bufs=2) as outp,
            tc.tile_pool(name="ps", bufs=3, space="PSUM") as ps,
            tc.tile_pool(name="po", bufs=1, space="PSUM") as po,
            tc.tile_pool(name="psm", bufs=1, space="PSUM") as psm,
        ):
            ones_col_f = const.tile([128, 1], F32)
            nc.vector.memset(ones_col_f[:], 1.0)
            ones_col = const.tile([128, 1], F32R)
            nc.vector.tensor_copy(ones_col[:], ones_col_f[:])
            ones_row_f = const.tile([1, 128], F32)
            nc.vector.memset(ones_row_f[:], 1.0)
            ones_row = const.tile([1, 128], F32R)
            nc.vector.tensor_copy(ones_row[:], ones_row_f[:])

            KTf = big.tile([128, M], F32)      # raw f32 loads
            QTf = big.tile([128, NLOC], F32)
            VSf = big.tile([128, M], F32)
            KT = big.tile([128, M], F32R)      # f32r operands for PE
            QT = big.tile([128, NLOC], F32R)
            VS = big.tile([128, M], F32R)      # V chunk mc at cols [mc*128,(mc+1)*128)

            # loads: wide stripes, casts: KT/QT on DVE, VS on ScalarE (keeps
            # each matmul's first cross-engine wait on a single semaphore)
            for c in range(M // WIDE):
                sl = slice(c * WIDE, (c + 1) * WIDE)
                nc.sync.dma_start(KTf[:, sl], KT_d[:, sl])
                nc.vector.tensor_copy(KT[:, sl], KTf[:, sl])
                nc.sync.dma_start(
                    VSf[:, sl].rearrange("p (c v) -> p c v", v=128),
                    V_r[:, c * 8 : (c + 1) * 8, :],
                )
                nc.scalar.copy(VS[:, sl], VSf[:, sl])
            nc.sync.dma_start(QTf[:], QT_d[:])
            nc.vector.tensor_copy(QT[:], QTf[:])

            for nt in range(NTILES):
                qsl = QT[:, nt * NT : (nt + 1) * NT]
                o_ps = po.tile([128, NT], F32, tag="po")
                s_ps = psm.tile([1, NT], F32, tag="psm")
                for g in range(NGRP):
                    sp = ps.tile([128, GRP * NT], F32, tag="sp")
                    for j in range(GRP):
                        mc = g * GRP + j
                        nc.tensor.matmul(
                            sp[:, j * NT : (j + 1) * NT],
                            KT[:, mc * 128 : (mc + 1) * 128],
                            qsl,
                            start=True,
                            stop=True,
                        )
                    et = etp.tile([128, GRP * NT], F32R, tag="et")
                    nc.scalar.activation(et[:], sp[:], EXP, scale=SCALE)
                    for j in range(GRP):
                        mc = g * GRP + j
                        ets = et[:, j * NT : (j + 1) * NT]
                        nc.tensor.matmul(
                            s_ps[:],
                            ones_col[:],
                            ets,
                            start=(mc == 0),
                            stop=(mc == NMC - 1),
                            skip_group_check=True,
                        )
                        nc.tensor.matmul(
                            o_ps[:],
                            VS[:, mc * 128 : (mc + 1) * 128],
                            ets,
                            start=(mc == 0),
                            stop=(mc == NMC - 1),
                            skip_group_check=True,
                        )

                # normalize: O^T / sums  (sums vary along free dim -> broadcast
                # across partitions with a rank-1 ones outer-product matmul)
                rec = outp.tile([1, NT], F32R, tag="rec")
                with nc.allow_low_precision(reason="f32r reciprocal, ~19-bit mantissa"):
                    nc.vector.reciprocal(rec[:], s_ps[:])
                bc_ps = ps.tile([128, NT], F32, tag="sp")
                nc.tensor.matmul(
                    bc_ps[:], ones_row[:], rec[:], start=True, stop=True
                )
                bc_sb = outp.tile([128, NT], F32, tag="bc")
                nc.vector.tensor_copy(bc_sb[:], bc_ps[:])
                o_sb = outp.tile([128, NT], F32, tag="osb")
                nc.vector.tensor_mul(o_sb[:], o_ps[:], bc_sb[:])
                nc.sync.dma_start(OT_d[:, nt * NT : (nt + 1) * NT], o_sb[:])

    return nc


def _fix_multiwaits(nc):
    """Walrus encodes at most one sem-wait on Matmult/Activation/DMACopy
    structs. Tile emits redundant same-engine waits (engines complete
    in order; the HW DRAIN covers intra-engine output hazards) - drop
    them so every such instruction carries a single wait."""
    eng_sem = {
        "EngineType.Activation": "Activation",
        "EngineType.PE": "PE",
        "EngineType.DVE": "DVE",
        "EngineType.Pool": "Pool",
        "EngineType.SP": "SP",
    }
    fn = nc.m.functions[0]
    leftover = []
    for blk in fn.blocks:
        for i in blk.instructions:
            si = getattr(i, "sync_info", None)
            if not si or not si.on_wait or len(si.on_wait) < 2:
                continue
            own = eng_sem.get(str(getattr(i, "engine", "")), "???")
            keep = [w for w in si.on_wait if not w.ant_name.startswith(own + "_")]
            if len(keep) < len(si.on_wait) and len(keep) <= 1:
                si.on_wait = keep
            elif len(si.on_wait) > 1:
                leftover.append((blk, i))
    # move extra waits onto standalone same-engine NoOps inserted before
    for blk, i in leftover:
        si = i.sync_info
        extra, keep = list(si.on_wait[:-1]), [si.on_wait[-1]]
        idx = next(k for k, x in enumerate(blk.instructions) if x.name == i.name)
        nops = []
        for w_i, w in enumerate(extra):
            nop = mybir.InstNoOp(name=f"W-{i.name}-{w_i}", ins=[], outs=[])
            nop.engine = i.engine
            nsi = mybir.SyncInfo(on_wait=[w], on_update=[])
            nop.sync_info = nsi
            nops.append(nop)
        blk.instructions[idx:idx] = nops
        si.on_wait = keep


_NC = None


def kernel(Q, K, V):
    global _NC, LAST_RESULT
    Q = np.asarray(Q, dtype=np.float32)
    K = np.asarray(K, dtype=np.float32)
    V = np.ascontiguousarray(np.asarray(V, dtype=np.float32))
    KT = np.ascontiguousarray(K.T)
    if _NC is None:
        _NC = build()
        _fix_multiwaits(_NC)
    in_maps = [
        {
            "QT": np.ascontiguousarray(Q[c * NLOC : (c + 1) * NLOC].T),
            "KT": KT,
            "V": V,
        }
        for c in range(NCORES)
    ]
    if TRACE:
        _install_ntff_hook()
    res = run_bass_kernel_spmd(
        _NC, in_maps, core_ids=list(range(NCORES)), trace=TRACE
    )
    LAST_RESULT = {
        "exec_time_ns": res.exec_time_ns,
        "mean_exec_time_ns": res.mean_exec_time_ns,
        "trace": res.instructions_and_trace,
        "profile_json": res.profile_json,
    }
    out = np.concatenate([r["OT"].T for r in res.results], axis=0)
    return np.ascontiguousarray(out.astype(np.float32))


def _install_ntff_hook():
    """Shim the missing antenv.axon_hooks module so run_bass_kernel_spmd's
    trace path can drive NTFF capture through libaxon_pjrt.so directly."""
    import sys
    import types

    try:
        from antenv.axon_hooks import get_axon_ntff_profile_hook  # noqa: F401
        return
    except ImportError:
        pass
    sys.path.insert(0, "/root/.axon_site")
    from trn_agent_boot.trn_boot import _ntff_profile_via_ctypes

    hook = _ntff_profile_via_ctypes("/opt/axon/libaxon_pjrt.so")
    mod = types.ModuleType("antenv.axon_hooks")
    mod.get_axon_ntff_profile_hook = lambda: hook
    mod.set_axon_ntff_profile_hook = lambda h: None
    sys.modules["antenv.axon_hooks"] = mod
